# revision 52
# baseline (speedup 1.0000x reference)
"""Modulated conv2d (StyleGAN-2 style, B=16 C=128 HxW=128x128 K=3) on 8 TRN2
NeuronCores, data-parallel over batch (2 samples/core), ~140 us HW time.

The style modulation + demodulation (a ~3 MFLOP computation) is folded into
the weights ON HOST, so the device kernel is a pure per-sample conv whose
matmul stream runs at the bf16 PE roofline (576 x N=512 taps, ~217 ns each,
~125 us, gapless). Everything else is arranged around that stream:

  per core:
    1. warm-up: dummy matmuls on a scratch region (contents irrelevant,
       results discarded) keep the PE busy from kernel start so the HAM
       clock-gate reaches K=8/8 (2.4 GHz) before the conv stream begins —
       the gate needs ~3.4 us of CONTINUOUS busy, so the warm-up must bridge
       exactly until the first input data lands
    2. input DMA (SP queue, FIFO): the early DMA phase is packet-count
       limited (~100 x 2-10KB lines/us across 16 engines), so the critical
       first transfer is one fused 128-line "head" = sample-0 weights +
       first 6 padded x rows; then big x chunks; SBUF layout [wm0|xs|wm1]
       keeps the head contiguous
    3. conv: per 4-row output block, 9 tap matmuls (K=C_in, M=C_out, N=512,
       the ISA max) accumulate fp32 in PSUM; the tap shift is a strided 3D
       rhs view into the padded 130x130 bf16 image — no im2col; 7 rotating
       PSUM banks
    4. evict: psum -> sbuf bf16 copy on DVE, 12 staging buffers
    5. DMA out (ACT-issued HWDGE): adjacent blocks pair into single 128-line
       transfers (halves the packet load); the final 4 rows are computed as
       R=3 + R=1 blocks and issued solo (2 via the idle SP queue) so the
       post-stream tail is short

Raw Bass with manual semaphores: this toolchain's walrus accepts only ONE
sync-wait command per instruction, so every engine-pair dependency is guarded
by an explicit single-wait `wait_ge`. Two DMAs that feed the same wait must
use SEPARATE semaphores — per-engine completion packets of back-to-back DMAs
interleave, so a shared counting semaphore can fire early.

Numerics: bf16 operands, fp32 accumulation, bf16 output (converted to fp32
on host); max rel err vs the fp32 jax reference ~3.7e-3.
"""

import sys

sys.path.insert(0, "/opt/trn_rl_repo")

import numpy as np

import concourse.bass as bass
from concourse import mybir
from concourse.bass_utils import run_bass_kernel_spmd

B, C, H, W, KS, WD = 16, 128, 128, 128, 3, 512
NCORES = 8
SPC = B // NCORES          # samples per core = 2
HP = H + 2                 # padded height/width = 130
NT = KS * KS               # 9 taps

R = 4                      # output rows per conv block (N = R*W = 512; PSUM bank cap)
NPS = 7                    # rotating conv PSUM banks
NOB = 12                   # output staging buffers
NB = H // R                # conv blocks per sample = 32
NWARM = 8                  # PE warm-up matmuls (N=512 each)
HEAD_ROWS = 6              # sample-0 x rows delivered with the weights head DMA


F32 = mybir.dt.float32
BF16 = mybir.dt.bfloat16
MULT = mybir.AluOpType.mult


def build_program():
    nc = bass.Bass(trn_type="TRN2", target_bir_lowering=False, debug=False)

    NW1 = NT * C                      # wmod elems per partition per sample
    NXS = SPC * HP * HP               # x elems per partition
    NHEAD = NW1 + HEAD_ROWS * HP      # head DMA elems per partition

    xpad_d = nc.dram_tensor("xpad", [SPC, C, HP, HP], BF16, kind="ExternalInput").ap()
    head_d = nc.dram_tensor("head", [C, NHEAD], BF16, kind="ExternalInput").ap()
    wmod1_d = nc.dram_tensor("wmod1", [C, NW1], BF16, kind="ExternalInput").ap()
    y_d = nc.dram_tensor("y", [SPC, C, H, W], BF16, kind="ExternalOutput").ap()

    # layout [wm0 | xs | wm1]: the head DMA (sample-0 weights + first x rows)
    # lands as a single 128-line transfer (early DMA is packet-count-limited)
    fused = nc.alloc_sbuf_tensor("fused", [C, NW1 + NXS + NW1], BF16).ap()
    wm = [fused[:, 0:NW1], fused[:, NW1 + NXS : NW1 + NXS + NW1]]
    xs = fused[:, NW1 : NW1 + NXS].rearrange("p (s h w) -> p s h w", s=SPC, h=HP)
    # per-sample 3D views: the conv rhs AP drops the size-1 sample dim
    xv = [fused[:, NW1 + s * HP * HP : NW1 + (s + 1) * HP * HP]
          .rearrange("p (h w) -> p h w", h=HP) for s in range(SPC)]
    outsb = nc.alloc_sbuf_tensor("outsb", [C, NOB, R * W], BF16).ap()
    warm = nc.alloc_sbuf_tensor("warm", [C, 512], BF16).ap()

    cps = [nc.alloc_psum_tensor(f"cps{j}", [C, R * W], F32).ap() for j in range(NPS)]
    wps = nc.alloc_psum_tensor("wps", [C, 512], F32).ap()

    sem_head = nc.alloc_semaphore("shead")
    sem_x = [nc.alloc_semaphore(f"sx{i}") for i in range(4)]
    sem_wm1 = nc.alloc_semaphore("swm1")
    sem_pe_blk = nc.alloc_semaphore("pe_blk")
    sem_dve_evict = nc.alloc_semaphore("dve_evict")
    sem_od = nc.alloc_semaphore("sod")   # counting: 16 per output DMA

    sem_x += [nc.alloc_semaphore(f"sx{i}") for i in (4, 5)]
    # x chunks after the head: (sample, row0, row1, sem idx), in issue order;
    # wmod_s1 is issued between sample-0 and sample-1 chunks
    XCHUNKS0 = [(0, HEAD_ROWS, 14, 0), (0, 14, 50, 1),
                (0, 50, 90, 4), (0, 90, HP, 5)]
    XCHUNKS1 = [(1, 0, 66, 2), (1, 66, HP, 3)]
    # (sample, block start row) -> semaphore to wait for (PE runs in order)
    WAITS = {(0, 0): sem_head, (0, 4): sem_x[0], (0, 12): sem_x[1],
             (0, 48): sem_x[4], (0, 88): sem_x[5],
             (1, 0): sem_x[2], (1, 64): sem_x[3]}
    # conv blocks: (sample, start row, n rows); last block split small so the
    # final evict+DMA chain after the last matmul is short
    BLOCKS = [(0, R * b, R) for b in range(NB)]
    BLOCKS += [(1, R * b, R) for b in range(NB - 1)]
    BLOCKS += [(1, H - R, R - 1), (1, H - 1, 1)]
    # output DMAs: adjacent R4 blocks pair into one 2-buffer transfer (the DMA
    # engine pool is packet-count-limited: one 128-line DMA per 2 blocks
    # halves the output packet load); the last 3 blocks go out solo.
    # entries: (last gb in transfer, sample, row0, n rows, ob, issuer)
    DMAS = []
    for k in range(31):
        s, r0, nr = BLOCKS[2 * k]
        DMAS.append((2 * k + 1, s, r0, 2 * R, (2 * k) % NOB, "act"))
    DMAS.append((62, 1, 120, R, 62 % NOB, "sp"))
    DMAS.append((63, 1, 124, R - 1, 63 % NOB, "sp"))
    DMAS.append((64, 1, 127, 1, 64 % NOB, "act"))

    with nc.Block() as blk:

        @blk.sync
        def _(eng):
            # single input queue, FIFO: head (sample-0 weights + first x rows),
            # then big x chunks — each transfer is 128 lines, and the early
            # DMA phase is packet-count-limited
            def xchunk(s, r0, r1, si):
                eng.dma_start(
                    out=xs[:, s : s + 1, r0:r1, :],
                    in_=xpad_d[s : s + 1, :, r0:r1, :],
                ).then_inc(sem_x[si], 16)

            eng.dma_start(
                out=fused[:, 0:NHEAD], in_=head_d
            ).then_inc(sem_head, 16)
            for c in XCHUNKS0:
                xchunk(*c)
            eng.dma_start(
                out=wm[1], in_=wmod1_d
            ).then_inc(sem_wm1, 16)
            for c in XCHUNKS1:
                xchunk(*c)
            # the two second-to-last output DMAs issue from here (SP is idle
            # by then) so ACT only has the final small block's DMA
            for gbl, s, r0, nr, ob, issuer in DMAS:
                if issuer != "sp":
                    continue
                eng.wait_ge(sem_dve_evict, gbl + 1)
                eng.dma_start(
                    out=y_d[s : s + 1, :, r0 : r0 + nr, :],
                    in_=outsb[:, ob : ob + (nr + R - 1) // R, 0 : min(nr, R) * W],
                ).then_inc(sem_od, 16)

        @blk.tensor
        def _(eng):
            # dummy matmuls on a scratch region (contents irrelevant — results
            # are discarded): keep the PE busy from kernel start so the HAM
            # clock-gate un-throttles before the real stream starts
            for i in range(NWARM):
                eng.matmul(out=wps, lhsT=warm[:, 0:C], rhs=warm[:, 0:512],
                           start=True, stop=True)

            def conv_block(s, r0, nr, gb):
                if (s, r0) in WAITS:
                    eng.wait_ge(WAITS[(s, r0)], 16)
                if gb >= NPS and (gb - NPS) % 4 == 0:
                    # covers bank reuse for blocks gb..gb+3 (reuse distance NPS)
                    eng.wait_ge(sem_dve_evict, gb - NPS + 4)
                for kh in range(KS):
                    for kw in range(KS):
                        t = kh * KS + kw
                        inst = eng.matmul(
                            out=cps[gb % NPS][:, 0 : nr * W],
                            lhsT=wm[s][:, t * C : (t + 1) * C],
                            rhs=xv[s][:, r0 + kh : r0 + kh + nr, kw : kw + W],
                            start=(t == 0),
                            stop=(t == NT - 1),
                        )
                inst.then_inc(sem_pe_blk, 1)

            for gb, (s, r0, nr) in enumerate(BLOCKS):
                if (s, r0) == (1, 0):
                    eng.wait_ge(sem_wm1, 16)
                conv_block(s, r0, nr, gb)

        @blk.vector
        def _(eng):
            # evictions: psum -> sbuf copy (demod scale folded into weights)
            for gb, (s, r0, nr) in enumerate(BLOCKS):
                eng.wait_ge(sem_pe_blk, gb + 1)
                if gb >= NOB:
                    # buffer gb%NOB was last used by block gb-NOB, which rides
                    # the ((gb-NOB)//2)-th paired output DMA
                    eng.wait_ge(sem_od, 16 * ((gb - NOB) // 2 + 1))
                eng.tensor_scalar(outsb[:, gb % NOB : gb % NOB + 1, 0 : nr * W],
                                  cps[gb % NPS][:, 0 : nr * W], 1.0,
                                  None, MULT).then_inc(sem_dve_evict, 1)

        @blk.scalar
        def _(eng):
            # output DMAs (two of the last three are issued by SP)
            for gbl, s, r0, nr, ob, issuer in DMAS:
                if issuer != "act":
                    continue
                eng.wait_ge(sem_dve_evict, gbl + 1)
                eng.dma_start(
                    out=y_d[s : s + 1, :, r0 : r0 + nr, :],
                    in_=outsb[:, ob : ob + (nr + R - 1) // R, 0 : min(nr, R) * W],
                ).then_inc(sem_od, 16)

    return nc


def _host_prep(x, w, weight, mod_w, mod_b):
    f = np.float32
    import ml_dtypes

    x = np.asarray(x, f)
    w = np.asarray(w, f)
    weight = np.asarray(weight, f)
    mod_w = np.asarray(mod_w, f)
    mod_b = np.asarray(mod_b, f)

    xpad = np.zeros((B, C, HP, HP), ml_dtypes.bfloat16)
    xpad[:, :, 1 : H + 1, 1 : W + 1] = x.astype(ml_dtypes.bfloat16)

    # style modulation + demodulation folded into the weights on host
    s = (w @ mod_w.T + mod_b).reshape(B, 1, C, 1, 1) + 1.0
    wgt = weight[None] * s                                    # [B, O, I, K, K]
    d = 1.0 / np.sqrt((wgt * wgt).sum(axis=(2, 3, 4)) + 1e-8)  # [B, O]
    wgt = wgt * d[:, :, None, None, None]
    # wmod[i, b, t*C + o] = wgt[b, o, i, kh, kw],  t = kh*3 + kw
    wT = np.ascontiguousarray(wgt.transpose(2, 0, 3, 4, 1)).reshape(C, B, NT * C)
    wT = wT.astype(ml_dtypes.bfloat16)

    in_maps = []
    for core in range(NCORES):
        s0 = SPC * core
        # head = sample-0 wmod + sample-0 padded rows 0:HEAD_ROWS
        head = np.empty((C, NT * C + HEAD_ROWS * HP), ml_dtypes.bfloat16)
        head[:, : NT * C] = wT[:, s0, :]
        head[:, NT * C :] = xpad[s0][:, :HEAD_ROWS, :].reshape(C, -1)
        in_maps.append({
            "xpad": np.ascontiguousarray(xpad[s0 : s0 + SPC]),
            "head": head,
            "wmod1": np.ascontiguousarray(wT[:, s0 + 1, :]),
        })
    return in_maps


_cached = {}


def kernel(x, w, weight, mod_w, mod_b):
    if "nc" not in _cached:
        _cached["nc"] = build_program()
    nc = _cached["nc"]
    in_maps = _host_prep(x, w, weight, mod_w, mod_b)
    res = run_bass_kernel_spmd(nc, in_maps, list(range(NCORES)))
    return np.concatenate(
        [res.results[i]["y"].astype(np.float32) for i in range(NCORES)], axis=0)


if __name__ == "__main__":
    from concourse.bass_utils import compile_bass_kernel
    import tempfile

    nc = build_program()
    d = tempfile.mkdtemp()
    neff = compile_bass_kernel(nc, d)
    print("compiled OK:", neff)


# revision 56
# speedup vs baseline: 1.0056x; 1.0056x over previous
"""Modulated conv2d (StyleGAN-2 style, B=16 C=128 HxW=128x128 K=3) on 8 TRN2
NeuronCores, data-parallel over batch (2 samples/core), ~140 us HW time.

The style modulation + demodulation (a ~3 MFLOP computation) is folded into
the weights ON HOST, so the device kernel is a pure per-sample conv whose
matmul stream runs at the bf16 PE roofline (576 x N=512 taps, ~217 ns each,
~125 us, gapless). Everything else is arranged around that stream:

  per core:
    1. warm-up: dummy matmuls on a scratch region (contents irrelevant,
       results discarded) keep the PE busy from kernel start so the HAM
       clock-gate reaches K=8/8 (2.4 GHz) before the conv stream begins —
       the gate needs ~3.4 us of CONTINUOUS busy, so the warm-up must bridge
       exactly until the first input data lands
    2. input DMA (SP queue, FIFO): the early DMA phase is packet-count
       limited (~100 x 2-10KB lines/us across 16 engines), so the critical
       first transfer is one fused 128-line "head" = sample-0 weights +
       first 6 padded x rows; then big x chunks; SBUF layout [wm0|xs|wm1]
       keeps the head contiguous
    3. conv: per 4-row output block, 9 tap matmuls (K=C_in, M=C_out, N=512,
       the ISA max) accumulate fp32 in PSUM; the tap shift is a strided 3D
       rhs view into the padded 130x130 bf16 image — no im2col; 7 rotating
       PSUM banks
    4. evict: psum -> sbuf bf16 copy on DVE, 12 staging buffers
    5. DMA out (ACT-issued HWDGE): adjacent blocks pair into single 128-line
       transfers (halves the packet load); the final 4 rows are computed as
       R=3 + R=1 blocks and issued solo (2 via the idle SP queue) so the
       post-stream tail is short

Raw Bass with manual semaphores: this toolchain's walrus accepts only ONE
sync-wait command per instruction, so every engine-pair dependency is guarded
by an explicit single-wait `wait_ge`. Two DMAs that feed the same wait must
use SEPARATE semaphores — per-engine completion packets of back-to-back DMAs
interleave, so a shared counting semaphore can fire early.

Numerics: bf16 operands, fp32 accumulation, bf16 output (converted to fp32
on host); max rel err vs the fp32 jax reference ~3.7e-3.
"""

import sys

sys.path.insert(0, "/opt/trn_rl_repo")

import numpy as np

import concourse.bass as bass
from concourse import mybir
from concourse.bass_utils import run_bass_kernel_spmd

B, C, H, W, KS, WD = 16, 128, 128, 128, 3, 512
NCORES = 8
SPC = B // NCORES          # samples per core = 2
HP = H + 2                 # padded height/width = 130
NT = KS * KS               # 9 taps

R = 4                      # output rows per conv block (N = R*W = 512; PSUM bank cap)
NPS = 7                    # rotating conv PSUM banks
NOB = 12                   # output staging buffers
NB = H // R                # conv blocks per sample = 32
NWARM = 8                  # PE warm-up matmuls (N=512 each)
HEAD_ROWS = 6              # sample-0 x rows delivered with the weights head DMA


F32 = mybir.dt.float32
BF16 = mybir.dt.bfloat16
MULT = mybir.AluOpType.mult


def build_program():
    nc = bass.Bass(trn_type="TRN2", target_bir_lowering=False, debug=False)

    NW1 = NT * C                      # wmod elems per partition per sample
    NXS = SPC * HP * HP               # x elems per partition
    NHEAD = NW1 + HEAD_ROWS * HP      # head DMA elems per partition

    xpad_d = nc.dram_tensor("xpad", [SPC, C, HP, HP], BF16, kind="ExternalInput").ap()
    head_d = nc.dram_tensor("head", [C, NHEAD], BF16, kind="ExternalInput").ap()
    wmod1_d = nc.dram_tensor("wmod1", [C, NW1], BF16, kind="ExternalInput").ap()
    y_d = nc.dram_tensor("y", [SPC, C, H, W], BF16, kind="ExternalOutput").ap()

    # layout [wm0 | xs | wm1]: the head DMA (sample-0 weights + first x rows)
    # lands as a single 128-line transfer (early DMA is packet-count-limited)
    fused = nc.alloc_sbuf_tensor("fused", [C, NW1 + NXS + NW1], BF16).ap()
    wm = [fused[:, 0:NW1], fused[:, NW1 + NXS : NW1 + NXS + NW1]]
    xs = fused[:, NW1 : NW1 + NXS].rearrange("p (s h w) -> p s h w", s=SPC, h=HP)
    # per-sample 3D views: the conv rhs AP drops the size-1 sample dim
    xv = [fused[:, NW1 + s * HP * HP : NW1 + (s + 1) * HP * HP]
          .rearrange("p (h w) -> p h w", h=HP) for s in range(SPC)]
    outsb = nc.alloc_sbuf_tensor("outsb", [C, NOB, R * W], BF16).ap()
    warm = nc.alloc_sbuf_tensor("warm", [C, 512], BF16).ap()

    cps = [nc.alloc_psum_tensor(f"cps{j}", [C, R * W], F32).ap() for j in range(NPS)]
    wps = nc.alloc_psum_tensor("wps", [C, 512], F32).ap()

    sem_head = nc.alloc_semaphore("shead")
    sem_x = [nc.alloc_semaphore(f"sx{i}") for i in range(4)]
    sem_wm1 = nc.alloc_semaphore("swm1")
    sem_pe_blk = nc.alloc_semaphore("pe_blk")
    sem_dve_evict = nc.alloc_semaphore("dve_evict")
    sem_od = nc.alloc_semaphore("sod")   # counting: 16 per output DMA

    sem_x += [nc.alloc_semaphore(f"sx{i}") for i in (4, 5)]
    # x chunks after the head: (sample, row0, row1, sem idx), in issue order;
    # wmod_s1 is issued between sample-0 and sample-1 chunks
    XCHUNKS0 = [(0, HEAD_ROWS, 14, 0), (0, 14, 50, 1),
                (0, 50, 90, 4), (0, 90, HP, 5)]
    XCHUNKS1 = [(1, 0, 66, 2), (1, 66, HP, 3)]
    # (sample, block start row) -> semaphore to wait for (PE runs in order)
    WAITS = {(0, 0): sem_head, (0, 4): sem_x[0], (0, 12): sem_x[1],
             (0, 48): sem_x[4], (0, 88): sem_x[5],
             (1, 0): sem_x[2], (1, 64): sem_x[3]}
    # conv blocks: (sample, start row, n rows); last block split small so the
    # final evict+DMA chain after the last matmul is short
    BLOCKS = [(0, R * b, R) for b in range(NB)]
    BLOCKS += [(1, R * b, R) for b in range(NB - 1)]
    BLOCKS += [(1, H - R, R - 1), (1, H - 1, 1)]
    # output DMAs: adjacent R4 blocks pair into one 2-buffer transfer (the DMA
    # engine pool is packet-count-limited: one 128-line DMA per 2 blocks
    # halves the output packet load); the last 3 blocks go out solo.
    # entries: (last gb in transfer, sample, row0, n rows, ob, issuer)
    DMAS = []
    for k in range(31):
        s, r0, nr = BLOCKS[2 * k]
        DMAS.append((2 * k + 1, s, r0, 2 * R, (2 * k) % NOB, "act"))
    DMAS.append((62, 1, 120, R, 62 % NOB, "sp"))
    DMAS.append((63, 1, 124, R - 1, 63 % NOB, "sp"))
    DMAS.append((64, 1, 127, 1, 64 % NOB, "act"))

    with nc.Block() as blk:

        @blk.sync
        def _(eng):
            # single input queue, FIFO: head (sample-0 weights + first x rows),
            # then big x chunks — each transfer is 128 lines, and the early
            # DMA phase is packet-count-limited
            def xchunk(s, r0, r1, si):
                eng.dma_start(
                    out=xs[:, s : s + 1, r0:r1, :],
                    in_=xpad_d[s : s + 1, :, r0:r1, :],
                ).then_inc(sem_x[si], 16)

            eng.dma_start(
                out=fused[:, 0:NHEAD], in_=head_d
            ).then_inc(sem_head, 16)
            for c in XCHUNKS0:
                xchunk(*c)
            eng.dma_start(
                out=wm[1], in_=wmod1_d
            ).then_inc(sem_wm1, 16)
            for c in XCHUNKS1:
                xchunk(*c)
            # the two second-to-last output DMAs issue from here (SP is idle
            # by then) so ACT only has the final small block's DMA
            for gbl, s, r0, nr, ob, issuer in DMAS:
                if issuer != "sp":
                    continue
                eng.wait_ge(sem_dve_evict, gbl + 1)
                eng.dma_start(
                    out=y_d[s : s + 1, :, r0 : r0 + nr, :],
                    in_=outsb[:, ob : ob + (nr + R - 1) // R, 0 : min(nr, R) * W],
                ).then_inc(sem_od, 16)

        @blk.tensor
        def _(eng):
            # dummy matmuls on a scratch region (contents irrelevant — results
            # are discarded): keep the PE busy from kernel start so the HAM
            # clock-gate un-throttles before the real stream starts
            for i in range(NWARM):
                eng.matmul(out=wps, lhsT=warm[:, 0:C], rhs=warm[:, 0:512],
                           start=True, stop=True)

            def conv_block(s, r0, nr, gb):
                if (s, r0) in WAITS:
                    eng.wait_ge(WAITS[(s, r0)], 16)
                if gb >= NPS and (gb - NPS) % 4 == 0:
                    # covers bank reuse for blocks gb..gb+3 (reuse distance NPS)
                    eng.wait_ge(sem_dve_evict, gb - NPS + 4)
                for kh in range(KS):
                    for kw in range(KS):
                        t = kh * KS + kw
                        inst = eng.matmul(
                            out=cps[gb % NPS][:, 0 : nr * W],
                            lhsT=wm[s][:, t * C : (t + 1) * C],
                            rhs=xv[s][:, r0 + kh : r0 + kh + nr, kw : kw + W],
                            start=(t == 0),
                            stop=(t == NT - 1),
                        )
                inst.then_inc(sem_pe_blk, 1)

            for gb, (s, r0, nr) in enumerate(BLOCKS):
                if (s, r0) == (1, 0):
                    eng.wait_ge(sem_wm1, 16)
                conv_block(s, r0, nr, gb)

        @blk.vector
        def _(eng):
            # evictions: psum -> sbuf copy (demod scale folded into weights)
            for gb, (s, r0, nr) in enumerate(BLOCKS):
                eng.wait_ge(sem_pe_blk, gb + 1)
                if gb >= NOB:
                    # buffer gb%NOB was last used by block gb-NOB, which rides
                    # the ((gb-NOB)//2)-th paired output DMA
                    eng.wait_ge(sem_od, 16 * ((gb - NOB) // 2 + 1))
                eng.tensor_scalar(outsb[:, gb % NOB : gb % NOB + 1, 0 : nr * W],
                                  cps[gb % NPS][:, 0 : nr * W], 1.0,
                                  None, MULT).then_inc(sem_dve_evict, 1)

        @blk.scalar
        def _(eng):
            # output DMAs (two of the last three are issued by SP)
            for gbl, s, r0, nr, ob, issuer in DMAS:
                if issuer != "act":
                    continue
                eng.wait_ge(sem_dve_evict, gbl + 1)
                eng.dma_start(
                    out=y_d[s : s + 1, :, r0 : r0 + nr, :],
                    in_=outsb[:, ob : ob + (nr + R - 1) // R, 0 : min(nr, R) * W],
                ).then_inc(sem_od, 16)

    return nc


def _host_prep(x, w, weight, mod_w, mod_b):
    f = np.float32
    import ml_dtypes

    x = np.asarray(x, f)
    w = np.asarray(w, f)
    weight = np.asarray(weight, f)
    mod_w = np.asarray(mod_w, f)
    mod_b = np.asarray(mod_b, f)

    xpad = np.zeros((B, C, HP, HP), ml_dtypes.bfloat16)
    xpad[:, :, 1 : H + 1, 1 : W + 1] = x.astype(ml_dtypes.bfloat16)

    # style modulation + demodulation folded into the weights on host
    s = (w @ mod_w.T + mod_b).reshape(B, 1, C, 1, 1) + 1.0
    wgt = weight[None] * s                                    # [B, O, I, K, K]
    d = 1.0 / np.sqrt((wgt * wgt).sum(axis=(2, 3, 4)) + 1e-8)  # [B, O]
    wgt = wgt * d[:, :, None, None, None]
    # wmod[i, b, t*C + o] = wgt[b, o, i, kh, kw],  t = kh*3 + kw
    wT = np.ascontiguousarray(wgt.transpose(2, 0, 3, 4, 1)).reshape(C, B, NT * C)
    wT = wT.astype(ml_dtypes.bfloat16)

    in_maps = []
    for core in range(NCORES):
        s0 = SPC * core
        # head = sample-0 wmod + sample-0 padded rows 0:HEAD_ROWS
        head = np.empty((C, NT * C + HEAD_ROWS * HP), ml_dtypes.bfloat16)
        head[:, : NT * C] = wT[:, s0, :]
        head[:, NT * C :] = xpad[s0][:, :HEAD_ROWS, :].reshape(C, -1)
        in_maps.append({
            "xpad": np.ascontiguousarray(xpad[s0 : s0 + SPC]),
            "head": head,
            "wmod1": np.ascontiguousarray(wT[:, s0 + 1, :]),
        })
    return in_maps


_cached = {}


def kernel(x, w, weight, mod_w, mod_b):
    if "nc" not in _cached:
        _cached["nc"] = build_program()
    nc = _cached["nc"]
    in_maps = _host_prep(x, w, weight, mod_w, mod_b)
    res = run_bass_kernel_spmd(nc, in_maps, list(range(NCORES)))
    return np.concatenate(
        [res.results[i]["y"].astype(np.float32) for i in range(NCORES)], axis=0)


if __name__ == "__main__":
    from concourse.bass_utils import compile_bass_kernel
    import tempfile

    nc = build_program()
    d = tempfile.mkdtemp()
    neff = compile_bass_kernel(nc, d)
    print("compiled OK:", neff)


# revision 57
# speedup vs baseline: 1.0152x; 1.0095x over previous
"""Modulated conv2d (StyleGAN-2 style, B=16 C=128 HxW=128x128 K=3) on 8 TRN2
NeuronCores, data-parallel over batch (2 samples/core), ~140 us HW time.

The style modulation + demodulation (a ~3 MFLOP computation) is folded into
the weights ON HOST, so the device kernel is a pure per-sample conv whose
matmul stream runs at the bf16 PE roofline (576 x N=512 taps, ~217 ns each,
~125 us, gapless). Everything else is arranged around that stream:

  per core:
    1. warm-up: dummy matmuls on a scratch region (contents irrelevant,
       results discarded) keep the PE busy from kernel start so the HAM
       clock-gate reaches K=8/8 (2.4 GHz) before the conv stream begins —
       the gate needs ~3.4 us of CONTINUOUS busy, so the warm-up must bridge
       exactly until the first input data lands
    2. input DMA (SP queue, FIFO): the early DMA phase is packet-count
       limited (~100 x 2-10KB lines/us across 16 engines), so the critical
       first transfer is one fused 128-line "head" = sample-0 weights +
       first 6 padded x rows; then big x chunks; SBUF layout [wm0|xs|wm1]
       keeps the head contiguous
    3. conv: per 4-row output block, 9 tap matmuls (K=C_in, M=C_out, N=512,
       the ISA max) accumulate fp32 in PSUM; the tap shift is a strided 3D
       rhs view into the padded 130x130 bf16 image — no im2col; 7 rotating
       PSUM banks
    4. evict: psum -> sbuf bf16 copy on DVE, 12 staging buffers
    5. DMA out (ACT-issued HWDGE): adjacent blocks pair into single 128-line
       transfers (halves the packet load); the final 4 rows are computed as
       R=3 + R=1 blocks and issued solo (2 via the idle SP queue) so the
       post-stream tail is short

Raw Bass with manual semaphores: this toolchain's walrus accepts only ONE
sync-wait command per instruction, so every engine-pair dependency is guarded
by an explicit single-wait `wait_ge`. Two DMAs that feed the same wait must
use SEPARATE semaphores — per-engine completion packets of back-to-back DMAs
interleave, so a shared counting semaphore can fire early.

Numerics: bf16 operands, fp32 accumulation, bf16 output (converted to fp32
on host); max rel err vs the fp32 jax reference ~3.7e-3.
"""

import sys

sys.path.insert(0, "/opt/trn_rl_repo")

import numpy as np

import concourse.bass as bass
from concourse import mybir
from concourse.bass_utils import run_bass_kernel_spmd

B, C, H, W, KS, WD = 16, 128, 128, 128, 3, 512
NCORES = 8
SPC = B // NCORES          # samples per core = 2
HP = H + 2                 # padded height/width = 130
NT = KS * KS               # 9 taps

R = 4                      # output rows per conv block (N = R*W = 512; PSUM bank cap)
NPS = 7                    # rotating conv PSUM banks
NOB = 12                   # output staging buffers
NB = H // R                # conv blocks per sample = 32
NWARM = 8                  # PE warm-up matmuls (N=512 each)
HEAD_ROWS = 6              # sample-0 x rows delivered with the weights head DMA


F32 = mybir.dt.float32
BF16 = mybir.dt.bfloat16
MULT = mybir.AluOpType.mult


def build_program():
    nc = bass.Bass(trn_type="TRN2", target_bir_lowering=False, debug=False)

    NW1 = NT * C                      # wmod elems per partition per sample
    NXS = SPC * HP * HP               # x elems per partition
    NHEAD = NW1 + HEAD_ROWS * HP      # head DMA elems per partition

    xpad_d = nc.dram_tensor("xpad", [SPC, C, HP, HP], BF16, kind="ExternalInput").ap()
    head_d = nc.dram_tensor("head", [C, NHEAD], BF16, kind="ExternalInput").ap()
    wmod1_d = nc.dram_tensor("wmod1", [C, NW1], BF16, kind="ExternalInput").ap()
    y_d = nc.dram_tensor("y", [SPC, C, H, W], BF16, kind="ExternalOutput").ap()

    # layout [wm0 | xs | wm1]: the head DMA (sample-0 weights + first x rows)
    # lands as a single 128-line transfer (early DMA is packet-count-limited)
    fused = nc.alloc_sbuf_tensor("fused", [C, NW1 + NXS + NW1], BF16).ap()
    wm = [fused[:, 0:NW1], fused[:, NW1 + NXS : NW1 + NXS + NW1]]
    xs = fused[:, NW1 : NW1 + NXS].rearrange("p (s h w) -> p s h w", s=SPC, h=HP)
    # per-sample 3D views: the conv rhs AP drops the size-1 sample dim
    xv = [fused[:, NW1 + s * HP * HP : NW1 + (s + 1) * HP * HP]
          .rearrange("p (h w) -> p h w", h=HP) for s in range(SPC)]
    outsb = nc.alloc_sbuf_tensor("outsb", [C, NOB, R * W], BF16).ap()
    warm = nc.alloc_sbuf_tensor("warm", [C, 512], BF16).ap()

    cps = [nc.alloc_psum_tensor(f"cps{j}", [C, R * W], F32).ap() for j in range(NPS)]
    wps = nc.alloc_psum_tensor("wps", [C, 512], F32).ap()

    sem_head = nc.alloc_semaphore("shead")
    sem_x = [nc.alloc_semaphore(f"sx{i}") for i in range(4)]
    sem_wm1 = nc.alloc_semaphore("swm1")
    sem_pe_blk = nc.alloc_semaphore("pe_blk")
    sem_dve_evict = nc.alloc_semaphore("dve_evict")
    sem_od = nc.alloc_semaphore("sod")   # counting: 16 per output DMA

    sem_x += [nc.alloc_semaphore(f"sx{i}") for i in (4, 5)]
    # x chunks after the head: (sample, row0, row1, sem idx), in issue order;
    # wmod_s1 is issued between sample-0 and sample-1 chunks
    XCHUNKS0 = [(0, HEAD_ROWS, 14, 0), (0, 14, 50, 1),
                (0, 50, 90, 4), (0, 90, HP, 5)]
    XCHUNKS1 = [(1, 0, 66, 2), (1, 66, HP, 3)]
    # (sample, block start row) -> semaphore to wait for (PE runs in order)
    WAITS = {(0, 0): sem_head, (0, 4): sem_x[0], (0, 12): sem_x[1],
             (0, 48): sem_x[4], (0, 88): sem_x[5],
             (1, 0): sem_x[2], (1, 64): sem_x[3]}
    # conv blocks: (sample, start row, n rows); last block split small so the
    # final evict+DMA chain after the last matmul is short
    BLOCKS = [(0, R * b, R) for b in range(NB)]
    BLOCKS += [(1, R * b, R) for b in range(NB - 1)]
    BLOCKS += [(1, H - R, R - 1), (1, H - 1, 1)]
    # output DMAs: adjacent R4 blocks pair into one 2-buffer transfer (the DMA
    # engine pool is packet-count-limited: one 128-line DMA per 2 blocks
    # halves the output packet load); the last 3 blocks go out solo.
    # entries: (last gb in transfer, sample, row0, n rows, ob, issuer)
    DMAS = []
    for k in range(31):
        s, r0, nr = BLOCKS[2 * k]
        DMAS.append((2 * k + 1, s, r0, 2 * R, (2 * k) % NOB, "act"))
    DMAS.append((62, 1, 120, R, 62 % NOB, "sp"))
    DMAS.append((63, 1, 124, R - 1, 63 % NOB, "sp"))
    DMAS.append((64, 1, 127, 1, 64 % NOB, "act"))

    with nc.Block() as blk:

        @blk.sync
        def _(eng):
            # single input queue, FIFO: head (sample-0 weights + first x rows),
            # then big x chunks — each transfer is 128 lines, and the early
            # DMA phase is packet-count-limited
            def xchunk(s, r0, r1, si):
                eng.dma_start(
                    out=xs[:, s : s + 1, r0:r1, :],
                    in_=xpad_d[s : s + 1, :, r0:r1, :],
                ).then_inc(sem_x[si], 16)

            eng.dma_start(
                out=fused[:, 0:NHEAD], in_=head_d
            ).then_inc(sem_head, 16)
            for c in XCHUNKS0:
                xchunk(*c)
            eng.dma_start(
                out=wm[1], in_=wmod1_d
            ).then_inc(sem_wm1, 16)
            for c in XCHUNKS1:
                xchunk(*c)
            # the two second-to-last output DMAs issue from here (SP is idle
            # by then) so ACT only has the final small block's DMA
            for gbl, s, r0, nr, ob, issuer in DMAS:
                if issuer != "sp":
                    continue
                eng.wait_ge(sem_dve_evict, gbl + 1)
                eng.dma_start(
                    out=y_d[s : s + 1, :, r0 : r0 + nr, :],
                    in_=outsb[:, ob : ob + (nr + R - 1) // R, 0 : min(nr, R) * W],
                ).then_inc(sem_od, 16)

        @blk.tensor
        def _(eng):
            # dummy matmuls on a scratch region (contents irrelevant — results
            # are discarded): keep the PE busy from kernel start so the HAM
            # clock-gate un-throttles before the real stream starts
            for i in range(NWARM):
                eng.matmul(out=wps, lhsT=warm[:, 0:C], rhs=warm[:, 0:512],
                           start=True, stop=True)

            def conv_block(s, r0, nr, gb):
                if (s, r0) in WAITS:
                    eng.wait_ge(WAITS[(s, r0)], 16)
                if gb >= NPS and (gb - NPS) % 4 == 0:
                    # covers bank reuse for blocks gb..gb+3 (reuse distance NPS)
                    eng.wait_ge(sem_dve_evict, gb - NPS + 4)
                for kh in range(KS):
                    for kw in range(KS):
                        t = kh * KS + kw
                        inst = eng.matmul(
                            out=cps[gb % NPS][:, 0 : nr * W],
                            lhsT=wm[s][:, t * C : (t + 1) * C],
                            rhs=xs[:, s : s + 1, r0 + kh : r0 + kh + nr,
                                   kw : kw + W],
                            start=(t == 0),
                            stop=(t == NT - 1),
                        )
                inst.then_inc(sem_pe_blk, 1)

            for gb, (s, r0, nr) in enumerate(BLOCKS):
                if (s, r0) == (1, 0):
                    eng.wait_ge(sem_wm1, 16)
                conv_block(s, r0, nr, gb)

        @blk.vector
        def _(eng):
            # evictions: psum -> sbuf copy (demod scale folded into weights)
            for gb, (s, r0, nr) in enumerate(BLOCKS):
                eng.wait_ge(sem_pe_blk, gb + 1)
                if gb >= NOB:
                    # buffer gb%NOB was last used by block gb-NOB, which rides
                    # the ((gb-NOB)//2)-th paired output DMA
                    eng.wait_ge(sem_od, 16 * ((gb - NOB) // 2 + 1))
                eng.tensor_scalar(outsb[:, gb % NOB : gb % NOB + 1, 0 : nr * W],
                                  cps[gb % NPS][:, 0 : nr * W], 1.0,
                                  None, MULT).then_inc(sem_dve_evict, 1)

        @blk.scalar
        def _(eng):
            # output DMAs (two of the last three are issued by SP)
            for gbl, s, r0, nr, ob, issuer in DMAS:
                if issuer != "act":
                    continue
                eng.wait_ge(sem_dve_evict, gbl + 1)
                eng.dma_start(
                    out=y_d[s : s + 1, :, r0 : r0 + nr, :],
                    in_=outsb[:, ob : ob + (nr + R - 1) // R, 0 : min(nr, R) * W],
                ).then_inc(sem_od, 16)

    return nc


def _host_prep(x, w, weight, mod_w, mod_b):
    f = np.float32
    import ml_dtypes

    x = np.asarray(x, f)
    w = np.asarray(w, f)
    weight = np.asarray(weight, f)
    mod_w = np.asarray(mod_w, f)
    mod_b = np.asarray(mod_b, f)

    xpad = np.zeros((B, C, HP, HP), ml_dtypes.bfloat16)
    xpad[:, :, 1 : H + 1, 1 : W + 1] = x.astype(ml_dtypes.bfloat16)

    # style modulation + demodulation folded into the weights on host
    s = (w @ mod_w.T + mod_b).reshape(B, 1, C, 1, 1) + 1.0
    wgt = weight[None] * s                                    # [B, O, I, K, K]
    d = 1.0 / np.sqrt((wgt * wgt).sum(axis=(2, 3, 4)) + 1e-8)  # [B, O]
    wgt = wgt * d[:, :, None, None, None]
    # wmod[i, b, t*C + o] = wgt[b, o, i, kh, kw],  t = kh*3 + kw
    wT = np.ascontiguousarray(wgt.transpose(2, 0, 3, 4, 1)).reshape(C, B, NT * C)
    wT = wT.astype(ml_dtypes.bfloat16)

    in_maps = []
    for core in range(NCORES):
        s0 = SPC * core
        # head = sample-0 wmod + sample-0 padded rows 0:HEAD_ROWS
        head = np.empty((C, NT * C + HEAD_ROWS * HP), ml_dtypes.bfloat16)
        head[:, : NT * C] = wT[:, s0, :]
        head[:, NT * C :] = xpad[s0][:, :HEAD_ROWS, :].reshape(C, -1)
        in_maps.append({
            "xpad": np.ascontiguousarray(xpad[s0 : s0 + SPC]),
            "head": head,
            "wmod1": np.ascontiguousarray(wT[:, s0 + 1, :]),
        })
    return in_maps


_cached = {}


def kernel(x, w, weight, mod_w, mod_b):
    if "nc" not in _cached:
        _cached["nc"] = build_program()
    nc = _cached["nc"]
    in_maps = _host_prep(x, w, weight, mod_w, mod_b)
    res = run_bass_kernel_spmd(nc, in_maps, list(range(NCORES)))
    return np.concatenate(
        [res.results[i]["y"].astype(np.float32) for i in range(NCORES)], axis=0)


if __name__ == "__main__":
    from concourse.bass_utils import compile_bass_kernel
    import tempfile

    nc = build_program()
    d = tempfile.mkdtemp()
    neff = compile_bass_kernel(nc, d)
    print("compiled OK:", neff)
